# revision 1
# baseline (speedup 1.0000x reference)
"""ConditionAwareAdaIN Trainium2 kernel.

Reference computation (B=16, C=256, L=1024, U=64, Q=64):
    nx    = InstanceNorm1d(x)                       # per-(b,c) stats over L
    A     = einsum('bu,cuq->bcq', u_i, W.reshape(2C,U,Q))
    style = einsum('bcq,bql->bcl', A, e_qid)
    gamma, beta = split(style + V@t + bias, 2, axis=1)
    out   = (1 + gamma) * nx + beta

Sharding: 2-way over batch x 4-way over channels -> 8 cores, each owning
8 samples x 64 channels (its slice of gamma AND beta rows of W/V/bias).

Per-core device kernel:
  stage 1: A[b,(q,c2)] via a batched matmul over K=u (host-pretransposed W),
           PSUM evacuated by ACT/DVE, bounced through DRAM into per-pair
           A_T tiles (q on partitions).
  stage 2: per sample-pair, block-diagonal style matmuls (K=128) + a K=3
           accumulating matmul folding V*t, bias and the "+1" of (1+gamma).
  norm:    bn_stats/bn_aggr per pair tile (2 samples x 64 ch = 128 rows),
           rstd folded into the multiplier during PSUM evacuation (ACT),
           then out = (x - mean) * M'' + beta on DVE.

float32r is used for all matmul operands (full-rate PE streaming, ~tf32
rounding; end-to-end rel err ~1e-4).
"""

import json

import numpy as np

for _p in ("/opt/trn_rl_repo", "/root/.axon_site/_ro/trn_rl_repo"):
    import sys as _sys

    if _p not in _sys.path:
        _sys.path.append(_p)

import concourse.bass as bass
import concourse.mybir as mybir
from concourse.tile import TileContext
from concourse.bass_utils import run_bass_kernel_spmd


def _split_sync_waits(raw: bytes, keep: int = 1) -> bytes:
    """Walrus in this env accepts at most one sync wait per TPB instruction.

    Tile packs several waits into sync_info.on_wait; re-emit the excess as
    standalone single-wait EventSemaphore instructions (what wait_ge emits)
    immediately before the instruction, in the same engine stream.
    """
    bir = json.loads(raw)
    n = 0
    for fn in bir["functions"]:
        for blk in fn["blocks"]:
            out = []
            for ins in blk["instructions"]:
                si = ins.get("sync_info")
                ws = si.get("on_wait") if si else None
                if ws and len(ws) > keep:
                    for w in ws[: len(ws) - keep]:
                        n += 1
                        out.append(
                            {
                                "debug": ins.get("debug", 0),
                                "engine": ins["engine"],
                                "ins": [],
                                "outs": [],
                                "name": f"evw-{n}",
                                "opcode": "EventSemaphore",
                                "sync_info": {"on_update": [], "on_wait": [w]},
                            }
                        )
                    si["on_wait"] = ws[len(ws) - keep :]
                out.append(ins)
            blk["instructions"] = out
    return json.dumps(bir).encode()


class _Bass(bass.Bass):
    def to_json_bytes(self) -> bytes:
        return _split_sync_waits(super().to_json_bytes())


B, C, L = 16, 256, 1024
U, Q = 64, 64
EPS = 1e-5
N_CORES = 8
BG, CG = 2, 4          # batch groups x channel groups
BPC = B // BG          # samples per core = 8
CPC = C // CG          # channels per core = 64
NPAIR = BPC // 2       # sample pairs per core = 4

FP32 = mybir.dt.float32

_CACHE = {}


def _build_nc(detect_races: bool = True):
    nc = _Bass(detect_race_conditions=detect_races)

    # xe: per pair, rows = (2 samples x 64 ch/q), cols = [x (1024) | e (1024)]
    xe_in = nc.dram_tensor("xe_s", [NPAIR, 128, 2 * L], FP32, kind="ExternalInput")
    # wt2 columns: [ui (8) | W-permuted (4096)] -- one tensor, one DMA per half
    wt_in = nc.dram_tensor("wt2", [128, BPC + 4096], FP32, kind="ExternalInput")
    # sm: [r2 (4 pairs x 1024) | l2 (256)] on 3 partitions
    sm_in = nc.dram_tensor("sm2", [3, NPAIR * L + 256], FP32, kind="ExternalInput")
    out_d = nc.dram_tensor("out_s", [BPC, CPC, L], FP32, kind="ExternalOutput")
    # DRAM bounce for the A redistribute: partition-offset SBUF DMA sources
    # flat-decode the offset into the free dim, so go through DRAM instead.
    a_dram = nc.dram_tensor("a_scratch", [BPC, Q, 2, CPC], FP32)

    AF = mybir.ActivationFunctionType
    OP = mybir.AluOpType
    F32R = mybir.dt.float32r

    with TileContext(nc) as tc:
        with (
            tc.tile_pool(name="persist", bufs=1) as persist,
            tc.tile_pool(name="xe", bufs=4) as xe,
            tc.tile_pool(name="work", bufs=4) as work,
            tc.tile_pool(name="stat", bufs=8) as stat,
            tc.tile_pool(name="ps", bufs=4, space="PSUM") as ps,
        ):
            sm = persist.tile([3, NPAIR * L + 256], F32R, tag="sm")
            nc.sync.dma_start(out=sm, in_=sm_in[:, :].bitcast(F32R))
            r2 = sm[:, 0 : NPAIR * L].rearrange("k (s l) -> k s l", s=NPAIR)
            l2 = sm[:, NPAIR * L : NPAIR * L + 256]
            # wt in two halves so stage-1 (qh=0) can start after the first one;
            # each half carries its own ui rows in cols 0:BPC
            wt = persist.tile([128, BPC + 4096], F32R, tag="wt")
            nc.sync.dma_start(out=wt[0:64, :], in_=wt_in[0:64, :].bitcast(F32R))
            nc.sync.dma_start(out=wt[64:128, :], in_=wt_in[64:128, :].bitcast(F32R))
            ui = wt[:, 0:BPC]

            a_sb = persist.tile([BPC, Q * 128], FP32, tag="a_sb")
            # block-diagonal lhsT, all pairs in one tile; per pair s the
            # free col = gb*128 + sp*64 + c':
            #   lt_all[sp*64+q, s, gb*128+sp'*64+c'] = A[2s+sp, gb*64+c', q] iff sp'==sp
            lt_all = persist.tile([128, NPAIR, 256], F32R, tag="lt_all")
            nc.gpsimd.memset(lt_all[:, :, :].bitcast(FP32), 0.0)
            eps_t = persist.tile([128, 1], FP32, tag="eps")
            nc.vector.memset(eps_t, EPS)

            # PE warm-up: dummy matmuls on early-arriving tiles keep the PE
            # HAM ramping while wt streams in, so stage-1 runs at full clock.
            for wu in range(4):
                pw = ps.tile([BPC, 512], FP32, tag="ps", name=f"pw{wu}")
                nc.tensor.matmul(
                    pw, lhsT=sm[:, 0:BPC], rhs=sm[:, 0:512], start=True, stop=True
                )

            # ---- stage 1: A[b, (q, c2)] = sum_u u_i[b,u] * Wr[c2,u,q] ----
            # wt rows (qh,u); free (ql, c2).  8 matmul groups of (8, 1024).
            for g in range(8):
                qh, ns = divmod(g, 4)
                pa = ps.tile([BPC, 1024], FP32, tag="ps", name=f"pa{g}")
                ks = slice(qh * 64, qh * 64 + 64)
                for h in range(2):
                    nc.tensor.matmul(
                        pa[:, h * 512 : (h + 1) * 512],
                        lhsT=ui[ks, :],
                        rhs=wt[ks, BPC + ns * 1024 + h * 512 : BPC + ns * 1024 + (h + 1) * 512],
                        start=True,
                        stop=True,
                    )
                dst = a_sb[:, qh * 4096 + ns * 1024 : qh * 4096 + (ns + 1) * 1024]
                if g % 2 == 0:
                    nc.scalar.activation(out=dst, in_=pa, func=AF.Copy)
                else:
                    nc.vector.tensor_copy(out=dst, in_=pa)

            # ---- redistribute A into per-pair A_T tiles (via DRAM) ----
            nc.sync.dma_start(out=a_dram[:, :, :, :], in_=a_sb[:, :])
            # two fills: one per sample-in-pair position (even b / odd b);
            # source order (q, s, gb, c) matches the dest AP flattening
            lt4 = lt_all.rearrange("p s (gb c) -> p s gb c", gb=2)
            av = a_dram.rearrange("(s two) q gb c -> two q s gb c", two=2)
            for sp in range(2):
                rows = slice(sp * 64, sp * 64 + 64)
                for gb in range(2):
                    nc.sync.dma_start(
                        out=lt4[rows, :, gb, sp * 64 : sp * 64 + 64],
                        in_=av[sp, :, :, gb, :].bitcast(F32R),
                    )

            # ---- stage 2 + norm, per sample pair ----
            for s in range(NPAIR):
                xet = xe.tile([128, 2 * L], F32R, tag="xe")
                nc.sync.dma_start(
                    out=xet[:, 0:L], in_=xe_in[s, :, 0:L].bitcast(F32R)
                )
                nc.sync.dma_start(
                    out=xet[:, L : 2 * L], in_=xe_in[s, :, L : 2 * L].bitcast(F32R)
                )
                xt = xet[:, 0:L].bitcast(FP32)
                et = xet[:, L : 2 * L]

                st = stat.tile([128, 2, 6], FP32, tag="st")
                nc.vector.bn_stats(st[:, 0, :], xt[:, 0:512])
                nc.vector.bn_stats(st[:, 1, :], xt[:, 512:1024])
                mv = stat.tile([128, 2], FP32, tag="mv")
                nc.vector.bn_aggr(mv, st)
                rstd = stat.tile([128, 1], FP32, tag="rstd")
                nc.scalar.activation(
                    out=rstd, in_=mv[:, 1:2], func=AF.Sqrt, bias=eps_t, scale=1.0
                )
                nc.vector.reciprocal(rstd, rstd)

                msb = work.tile([128, L], FP32, tag="m")
                ot = work.tile([128, L], FP32, tag="o")
                pm = ps.tile([128, L], FP32, tag="ps", name=f"pm{s}")
                pb = ps.tile([128, L], FP32, tag="ps", name=f"pb{s}")
                for n in range(2):
                    cols = slice(n * 512, (n + 1) * 512)
                    pmh = pm[:, cols]
                    pbh = pb[:, cols]
                    nc.tensor.matmul(
                        pmh,
                        lhsT=lt_all[:, s, 0:128],
                        rhs=et[:, cols],
                        start=True,
                        stop=False,
                    )
                    nc.tensor.matmul(
                        pmh,
                        lhsT=l2[:, 0:128],
                        rhs=r2[:, s, cols],
                        start=False,
                        stop=True,
                    )
                    nc.tensor.matmul(
                        pbh,
                        lhsT=lt_all[:, s, 128:256],
                        rhs=et[:, cols],
                        start=True,
                        stop=False,
                    )
                    nc.tensor.matmul(
                        pbh,
                        lhsT=l2[:, 128:256],
                        rhs=r2[:, s, cols],
                        start=False,
                        stop=True,
                    )
                    # evacuate + fold rstd:  M'' = (1+gamma) * rstd
                    nc.scalar.activation(
                        out=msb[:, cols], in_=pmh, func=AF.Copy, scale=rstd
                    )
                    # ot = (x - mean) * M''   (fused),  then += beta
                    nc.vector.scalar_tensor_tensor(
                        out=ot[:, cols],
                        in0=xt[:, cols],
                        scalar=mv[:, 0:1],
                        in1=msb[:, cols],
                        op0=OP.subtract,
                        op1=OP.mult,
                    )
                    nc.vector.tensor_add(
                        out=ot[:, cols], in0=ot[:, cols], in1=pbh
                    )

                od = out_d.rearrange("b c (h l) -> b c h l", h=2)
                for n in range(2):
                    nc.sync.dma_start(
                        out=od[2 * s : 2 * s + 2, :, n, :],
                        in_=ot[:, n * 512 : (n + 1) * 512],
                    )

    return nc


def _prep_core_inputs(core, x, u_i, e_qid, t, W, V, bias):
    bg, cg = divmod(core, CG)
    bs = slice(bg * BPC, (bg + 1) * BPC)
    rg = slice(cg * CPC, (cg + 1) * CPC)
    rb = slice(C + cg * CPC, C + (cg + 1) * CPC)

    # xe: (NPAIR, 128, 2048) = [x pair rows | e pair rows]
    xp = x[bs, rg, :].reshape(NPAIR, 128, L)
    ep = e_qid[bs].reshape(NPAIR, 128, L)
    xe = np.concatenate([xp, ep], axis=2)

    w2 = np.concatenate([W[rg], W[rb]], axis=0)          # (128, 4096) c2=[g|b]
    wr = w2.reshape(128, U, 2, 32)                       # [c2, u, qh, ql]
    wt2 = np.ascontiguousarray(wr.transpose(2, 1, 3, 0)).reshape(128, 4096)

    ui_s = np.ascontiguousarray(u_i[bs].T)               # (64, 8)
    ui2 = np.concatenate([ui_s, ui_s], axis=0)           # (128, 8)
    wt2 = np.concatenate([ui2, wt2], axis=1)             # (128, 8+4096)

    vg, vb = V[rg, 0], V[rb, 0]
    bgm, bbt = bias[rg], bias[rb]
    l2 = np.zeros((3, 256), np.float32)
    l2[0, 0:64] = vg
    l2[1, 64:128] = vg
    l2[2, 0:64] = 1.0 + bgm
    l2[2, 64:128] = 1.0 + bgm
    l2[0, 128:192] = vb
    l2[1, 192:256] = vb
    l2[2, 128:192] = bbt
    l2[2, 192:256] = bbt

    r2 = np.empty((3, NPAIR, L), np.float32)
    for s in range(NPAIR):
        r2[0, s] = t[bg * BPC + 2 * s, 0]
        r2[1, s] = t[bg * BPC + 2 * s + 1, 0]
    r2[2] = 1.0
    sm = np.concatenate([r2.reshape(3, NPAIR * L), l2], axis=1)

    return {
        "xe_s": np.ascontiguousarray(xe, dtype=np.float32),
        "wt2": wt2.astype(np.float32),
        "sm2": np.ascontiguousarray(sm, dtype=np.float32),
    }


def kernel(x, u_i, e_qid, t, W, V, bias):
    x = np.asarray(x, np.float32)
    u_i = np.asarray(u_i, np.float32)
    e_qid = np.asarray(e_qid, np.float32)
    t = np.asarray(t, np.float32)
    W = np.asarray(W, np.float32)
    V = np.asarray(V, np.float32)
    bias = np.asarray(bias, np.float32)

    if "nc" not in _CACHE:
        _CACHE["nc"] = _build_nc()
    nc = _CACHE["nc"]

    in_maps = [
        _prep_core_inputs(i, x, u_i, e_qid, t, W, V, bias) for i in range(N_CORES)
    ]
    results = run_bass_kernel_spmd(nc, in_maps, list(range(N_CORES))).results

    out = np.empty((B, C, L), np.float32)
    for i in range(N_CORES):
        bg, cg = divmod(i, CG)
        out[bg * BPC : (bg + 1) * BPC, cg * CPC : (cg + 1) * CPC, :] = results[i][
            "out_s"
        ]
    return out



# revision 2
# speedup vs baseline: 1.8096x; 1.8096x over previous
"""ConditionAwareAdaIN Trainium2 kernel.

Reference computation (B=16, C=256, L=1024, U=64, Q=64):
    nx    = InstanceNorm1d(x)                       # per-(b,c) stats over L
    A     = einsum('bu,cuq->bcq', u_i, W.reshape(2C,U,Q))
    style = einsum('bcq,bql->bcl', A, e_qid)
    gamma, beta = split(style + V@t + bias, 2, axis=1)
    out   = (1 + gamma) * nx + beta

Sharding: 8-way data-parallel over batch -> each core owns 2 samples with
all 256 channels.  The tiny conditioning contraction A = u_i @ W (34 MFLOP
total) is folded into the per-sample weights on the host, so no core needs
the 8 MiB W; everything on the wire is bf16 (rel err ~3e-3, well inside
the 2e-2 gate).

Per-core device kernel, 4 chunks of (sample, 128-channel half):
  params:  one K=66 matmul per chunk half computes gamma' = 1+gamma+bias
           (rows of lt fold V*t via an appended t-row and bias via a
           ones-row in the rhs) and likewise beta into PSUM.
  norm:    bn_stats/bn_aggr per chunk, rstd = 1/sqrt(var+eps).
  output:  M = gamma'*rstd (ACT evac), xm = (x-mean)*rstd... actually
           xm = (x-mean) folded with rstd via tensor_scalar 4x mode,
           o1 = xm*M (DVE 2x), then the PE accumulates o1 into the beta
           PSUM bank through an identity matmul (start=False), and the
           final ACT copy evacuates beta+o1 = out to bf16 for the DMA.
  A few warm-up matmuls at t=0 ramp the PE out of its low p-state while
  the input DMAs stream.
"""

import json

import numpy as np
import ml_dtypes

for _p in ("/opt/trn_rl_repo", "/root/.axon_site/_ro/trn_rl_repo"):
    import sys as _sys

    if _p not in _sys.path:
        _sys.path.append(_p)

import concourse.bass as bass
import concourse.mybir as mybir
from concourse.tile import TileContext
from concourse.bass_utils import run_bass_kernel_spmd


def _split_sync_waits(raw: bytes, keep: int = 1) -> bytes:
    """Walrus in this env accepts at most one sync wait per TPB instruction.

    Tile packs several waits into sync_info.on_wait; re-emit the excess as
    standalone single-wait EventSemaphore instructions (what wait_ge emits)
    immediately before the instruction, in the same engine stream.
    """
    bir = json.loads(raw)
    n = 0
    for fn in bir["functions"]:
        for blk in fn["blocks"]:
            out = []
            for ins in blk["instructions"]:
                si = ins.get("sync_info")
                ws = si.get("on_wait") if si else None
                if ws and len(ws) > keep:
                    for w in ws[: len(ws) - keep]:
                        n += 1
                        out.append(
                            {
                                "debug": ins.get("debug", 0),
                                "engine": ins["engine"],
                                "ins": [],
                                "outs": [],
                                "name": f"evw-{n}",
                                "opcode": "EventSemaphore",
                                "sync_info": {"on_update": [], "on_wait": [w]},
                            }
                        )
                    si["on_wait"] = ws[len(ws) - keep :]
                out.append(ins)
            blk["instructions"] = out
    return json.dumps(bir).encode()


class _Bass(bass.Bass):
    def to_json_bytes(self) -> bytes:
        return _split_sync_waits(super().to_json_bytes())


B, C, L = 16, 256, 1024
U, Q = 64, 64
K = Q + 2  # q rows + t row + ones row
EPS = 1e-5
N_CORES = 8
BPC = B // N_CORES     # samples per core = 2
NCHUNK = 4             # (sample, channel-half) chunks of 128 rows

FP32 = mybir.dt.float32
BF16 = mybir.dt.bfloat16
BF = ml_dtypes.bfloat16

_CACHE = {}


def _build_nc(detect_races: bool = True):
    nc = _Bass(detect_race_conditions=detect_races)

    x_in = nc.dram_tensor("x_s", [NCHUNK, 128, L], BF16, kind="ExternalInput")
    ea_in = nc.dram_tensor("ea_s", [K, BPC, L], BF16, kind="ExternalInput")
    lt_in = nc.dram_tensor("lt_s", [K, NCHUNK, 2, 128], BF16, kind="ExternalInput")
    id_in = nc.dram_tensor("id_s", [128, 128], BF16, kind="ExternalInput")
    out_d = nc.dram_tensor("out_s", [NCHUNK, 128, L], BF16, kind="ExternalOutput")

    AF = mybir.ActivationFunctionType
    OP = mybir.AluOpType

    with TileContext(nc) as tc:
        with (
            tc.tile_pool(name="persist", bufs=1) as persist,
            tc.tile_pool(name="xp", bufs=4) as xp,
            tc.tile_pool(name="mp", bufs=2) as mp,
            tc.tile_pool(name="xmp", bufs=2) as xmp,
            tc.tile_pool(name="o1p", bufs=2) as o1p,
            tc.tile_pool(name="fop", bufs=2) as fop,
            tc.tile_pool(name="stp", bufs=4) as stp,
            tc.tile_pool(name="ps", bufs=4, space="PSUM") as ps,
        ):
            wt = persist.tile([128, 512], BF16, tag="wt")
            nc.gpsimd.memset(wt, 0.0)
            eps_t = persist.tile([128, 1], FP32, tag="eps")
            nc.vector.memset(eps_t, EPS)

            ea = persist.tile([K, BPC, L], BF16, tag="ea")
            lt = persist.tile([K, NCHUNK, 2, 128], BF16, tag="lt")
            idt = persist.tile([128, 128], BF16, tag="idt")
            xts = [xp.tile([128, L], BF16, tag="x", name=f"x{c}") for c in range(NCHUNK)]

            nc.sync.dma_start(out=xts[0], in_=x_in[0])
            nc.sync.dma_start(out=ea, in_=ea_in[:, :, :])
            nc.sync.dma_start(out=lt, in_=lt_in[:, :, :, :])
            nc.sync.dma_start(out=idt, in_=id_in[:, :])
            for c in range(1, NCHUNK):
                nc.sync.dma_start(out=xts[c], in_=x_in[c])

            # PE p-state warm-up while inputs stream in.
            for wu in range(6):
                pw = ps.tile([128, L], FP32, tag="ps", name=f"pw{wu // 2}")
                nc.tensor.matmul(
                    pw[:, (wu % 2) * 512 : (wu % 2) * 512 + 512],
                    lhsT=wt[:, 0:128],
                    rhs=wt,
                    start=True,
                    stop=True,
                )

            for c in range(NCHUNK):
                s = c // 2
                xt = xts[c]

                bst = stp.tile([128, 2, 6], FP32, tag="bst")
                nc.vector.bn_stats(bst[:, 0, :], xt[:, 0:512])
                nc.vector.bn_stats(bst[:, 1, :], xt[:, 512:1024])
                mv = stp.tile([128, 2], FP32, tag="mv")
                nc.vector.bn_aggr(mv, bst)
                rstd = stp.tile([128, 1], FP32, tag="rstd")
                nc.scalar.activation(
                    out=rstd, in_=mv[:, 1:2], func=AF.Sqrt, bias=eps_t, scale=1.0
                )
                nc.vector.reciprocal(rstd, rstd)

                pg = ps.tile([128, L], FP32, tag="ps", name=f"pg{c}")
                pb = ps.tile([128, L], FP32, tag="ps", name=f"pb{c}")
                for h in range(2):
                    cs = slice(h * 512, (h + 1) * 512)
                    nc.tensor.matmul(
                        pg[:, cs], lhsT=lt[:, c, 0, :], rhs=ea[:, s, cs],
                        start=True, stop=True,
                    )
                    nc.tensor.matmul(
                        pb[:, cs], lhsT=lt[:, c, 1, :], rhs=ea[:, s, cs],
                        start=True, stop=False,
                    )

                # M = gamma' * rstd  (PSUM evac on ACT)
                mt = mp.tile([128, L], BF16, tag="m")
                nc.scalar.activation(out=mt, in_=pg, func=AF.Copy, scale=rstd)
                # xm = (x - mean)  (DVE 4x mode: all-bf16 SBUF)
                xm = xmp.tile([128, L], BF16, tag="xm")
                nc.vector.tensor_scalar(
                    out=xm, in0=xt, scalar1=mv[:, 0:1], scalar2=None, op0=OP.subtract
                )
                # o1 = xm * M  (DVE 2x)
                o1 = o1p.tile([128, L], BF16, tag="o1")
                nc.vector.tensor_tensor(out=o1, in0=xm, in1=mt, op=OP.mult)
                # beta PSUM += o1 via identity matmul
                for h in range(2):
                    cs = slice(h * 512, (h + 1) * 512)
                    nc.tensor.matmul(
                        pb[:, cs], lhsT=idt, rhs=o1[:, cs], start=False, stop=True
                    )
                # out = beta + o1  (final evac)
                fo = fop.tile([128, L], BF16, tag="fo")
                nc.scalar.activation(out=fo, in_=pb, func=AF.Copy)
                nc.sync.dma_start(out=out_d[c], in_=fo)

    return nc


def _prep_core_inputs(core, x, e_qid, t, A, V, bias):
    b0 = core * BPC

    x_s = np.ascontiguousarray(x[b0 : b0 + BPC].reshape(NCHUNK, 128, L)).astype(BF)

    ea = np.empty((K, BPC, L), np.float32)
    ea[0:Q] = e_qid[b0 : b0 + BPC].transpose(1, 0, 2)
    ea[Q] = t[b0 : b0 + BPC, 0, :]
    ea[Q + 1] = 1.0

    lt = np.empty((K, NCHUNK, 2, 128), np.float32)
    for c in range(NCHUNK):
        s, h = divmod(c, 2)
        rg = slice(h * 128, (h + 1) * 128)          # gamma rows c2
        rb = slice(C + h * 128, C + (h + 1) * 128)  # beta rows c2
        lt[0:Q, c, 0, :] = A[b0 + s, rg, :].T
        lt[0:Q, c, 1, :] = A[b0 + s, rb, :].T
        lt[Q, c, 0, :] = V[rg, 0]
        lt[Q, c, 1, :] = V[rb, 0]
        lt[Q + 1, c, 0, :] = 1.0 + bias[rg]
        lt[Q + 1, c, 1, :] = bias[rb]

    return {
        "x_s": x_s,
        "ea_s": ea.astype(BF),
        "lt_s": lt.astype(BF),
        "id_s": np.eye(128, dtype=np.float32).astype(BF),
    }


def kernel(x, u_i, e_qid, t, W, V, bias):
    x = np.asarray(x, np.float32)
    u_i = np.asarray(u_i, np.float32)
    e_qid = np.asarray(e_qid, np.float32)
    t = np.asarray(t, np.float32)
    W = np.asarray(W, np.float32)
    V = np.asarray(V, np.float32)
    bias = np.asarray(bias, np.float32)

    # Fold the conditioning contraction into per-sample style weights:
    # A[b, c2, q] = sum_u u_i[b, u] * W[c2, u*Q + q]
    A = np.einsum("bu,cuq->bcq", u_i, W.reshape(2 * C, U, Q), optimize=True)

    if "nc" not in _CACHE:
        _CACHE["nc"] = _build_nc()
    nc = _CACHE["nc"]

    in_maps = [
        _prep_core_inputs(i, x, e_qid, t, A, V, bias) for i in range(N_CORES)
    ]
    results = run_bass_kernel_spmd(nc, in_maps, list(range(N_CORES))).results

    out = np.empty((B, C, L), np.float32)
    for i in range(N_CORES):
        out[i * BPC : (i + 1) * BPC] = (
            np.asarray(results[i]["out_s"]).astype(np.float32).reshape(BPC, C, L)
        )
    return out
